# revision 13
# baseline (speedup 1.0000x reference)
"""Trainium2 Bass kernel for nn_Attention_20315195310831.

Fused attention block: q/k/v projections, per-head RMS-norm on q/k, masked
softmax with per-head gating, value residual, output projection.

Sharding over 8 NeuronCores: core = 4*b + grp handles batch b and heads
[4*grp, 4*grp+4). Each core computes its partial (attn_out + vx) @ Wo_slice;
the host sums the 4 partials per batch.

Layout strategy: everything is computed k/score-transposed (S^T[k, q]) so no
on-device transposes of the big score matrix are needed; softmax row-sums come
from a ones-column appended to V; the division by the softmax denominator and
the q/k RMS-norm scales are applied via rank-1 broadcast matmuls. The per-head
gate is folded into Wv (divide) and Wo (multiply) on the host.

All matmul operands are bf16 (PSUM accumulation stays fp32), inputs/outputs
are staged in DRAM as bf16; this halves DMA traffic and avoids the fp32r
short-pass penalty on diagonal mask blocks.

The kernel is specialized at build time to the observed attention_mask:
128x128 blocks that attend nowhere are skipped entirely (the causal 2x
saving falls out of this generically); partially masked blocks get a
deduplicated 0/1 pattern multiplied into exp(S) on the GPSIMD engine.
"""

import sys

sys.path.insert(0, "/opt/trn_rl_repo")

import ml_dtypes
import numpy as np

B, T, C = 2, 2048, 1024
H, D = 16, 64
EPS = 1e-5
SCALE = 1.0 / 8.0  # 1/sqrt(D)
NCORES = 8
HPC = 4  # heads per core
NG = 2  # head-pair groups per core
CB = C // 128  # contraction chunks
QT = 4  # q tiles of 512
QW = 512
TBLK = T // 128  # 128-blocks along T
BF16 = ml_dtypes.bfloat16

_CACHE = {}


def _analyze_mask(mask01):
    """mask01: bool [T, T], mask01[q, k] True = attend.

    Returns (plan, patterns):
      plan[j] = list of (kb, ql, qh, subs) in ascending kb for q-tile j, where
        [ql, qh) is the local (within 512) column range to compute and
        subs = [(qb_local, pat_idx)] lists 128-wide subblocks needing a
        multiplicative mask pattern.
      patterns: float32 [npat, 128, 128] multiplicative 0/1 masks in
        [k, q] orientation (applied to exp(S) post-activation).
    """
    pat_index = {}
    patterns = []

    def pat_id(block_qk):
        add = np.where(block_qk.T, 1.0, 0.0).astype(np.float32)
        key = add.tobytes()
        if key not in pat_index:
            pat_index[key] = len(patterns)
            patterns.append(add)
        return pat_index[key]

    plan = []
    for j in range(QT):
        entries = []
        for kb in range(TBLK):
            qbs = []
            for qb in range(4):
                blk = mask01[
                    (4 * j + qb) * 128 : (4 * j + qb + 1) * 128,
                    kb * 128 : (kb + 1) * 128,
                ]
                qbs.append(blk)
            anyb = [b.any() for b in qbs]
            if not any(anyb):
                continue
            lo = anyb.index(True)
            hi = 4 - anyb[::-1].index(True)
            entries.append([kb, lo, hi, qbs])
        if entries:
            # widen first entry to union range so the first PSUM-accumulation
            # matmul covers every column later matmuls will touch
            ulo = min(e[1] for e in entries)
            uhi = max(e[2] for e in entries)
            entries[0][1] = ulo
            entries[0][2] = uhi
        final = []
        for kb, lo, hi, qbs in entries:
            subs = []
            for qb in range(lo, hi):
                if not qbs[qb].all():
                    subs.append((qb, pat_id(qbs[qb])))
            final.append((kb, lo * 128, hi * 128, subs))
        plan.append(final)

    if not patterns:
        patterns.append(np.zeros((128, 128), np.float32))
    return plan, np.stack(patterns)


def _build_program(plan, npat, neg_bias):
    import concourse.bass as bass
    import concourse.mybir as mybir
    import concourse.tile as tile
    from concourse import bacc

    f32 = mybir.dt.float32
    f32r = mybir.dt.float32r
    bf16 = mybir.dt.bfloat16
    AF = mybir.ActivationFunctionType
    OP = mybir.AluOpType

    nc = bacc.Bacc(
        "TRN2",
        target_bir_lowering=False,
        debug=False,
        enable_asserts=False,
        num_devices=NCORES,
    )

    xT_d = nc.dram_tensor("xT", [C, T], bf16, kind="ExternalInput").ap()
    wq_d = nc.dram_tensor("wq", [128, 2048], bf16, kind="ExternalInput").ap()
    wk_d = nc.dram_tensor("wk", [128, 2048], bf16, kind="ExternalInput").ap()
    wv_d = nc.dram_tensor("wv", [128, 2048], bf16, kind="ExternalInput").ap()
    wo_d = nc.dram_tensor("wo", [128, 2048], bf16, kind="ExternalInput").ap()
    wqc_d = nc.dram_tensor("wq_col", [128, 1], f32, kind="ExternalInput").ap()
    wkc_d = nc.dram_tensor("wk_col", [128, 1], f32, kind="ExternalInput").ap()
    ones_row_d = nc.dram_tensor("ones_row", [1, 128], f32r, kind="ExternalInput").ap()
    sel2_d = nc.dram_tensor("sel2", [128, 2], bf16, kind="ExternalInput").ap()
    selT_d = nc.dram_tensor("selT", [2, 128], f32r, kind="ExternalInput").ap()
    ident_d = nc.dram_tensor("ident128", [128, 128], bf16, kind="ExternalInput").ap()
    pats_d = nc.dram_tensor("pats", [128, 128 * npat], bf16, kind="ExternalInput").ap()
    out_d = nc.dram_tensor("out", [T, C], bf16, kind="ExternalOutput").ap()

    with tile.TileContext(nc) as tc, \
         nc.allow_low_precision(reason="bf16 staging validated against fp32 reference"):
        with tc.tile_pool(name="pers", bufs=1) as pers:
            qT = [pers.tile([128, T], bf16, tag=f"qT{g}", name=f"qT{g}") for g in range(NG)]
            kT = [pers.tile([128, T], bf16, tag=f"kT{g}", name=f"kT{g}") for g in range(NG)]
            vT = [pers.tile([128, T], bf16, tag=f"vT{g}", name=f"vT{g}") for g in range(NG)]
            # per k-block: 4 heads x 65 cols of [V(64) | 1]; the ones column
            # accumulates the softmax denominators for free.
            vaug = [
                pers.tile([128, 65 * HPC], bf16, tag=f"vaug{kb}", name=f"vaug{kb}")
                for kb in range(TBLK)
            ]
            wo_sb = pers.tile([128, 2048], bf16, tag="wo_sb", name="wo_sb")
            wq_col = pers.tile([128, 1], f32, tag="wq_col_sb", name="wq_col_sb")
            wk_col = pers.tile([128, 1], f32, tag="wk_col_sb", name="wk_col_sb")
            ones_row = pers.tile([1, 128], f32r, tag="ones_row_sb", name="ones_row_sb")
            sel2 = pers.tile([128, 2], bf16, tag="sel2_sb", name="sel2_sb")
            selT = pers.tile([2, 128], f32r, tag="selT_sb", name="selT_sb")
            ident = pers.tile([128, 128], bf16, tag="ident_sb", name="ident_sb")
            pats = pers.tile([128, 128 * npat], bf16, tag="pats_sb", name="pats_sb")
            eps_col = pers.tile([128, 1], f32, tag="eps_col", name="eps_col")
            nb_col = pers.tile([128, 1], f32, tag="nb_col", name="nb_col")
            one_col = pers.tile([128, 1], bf16, tag="one_col", name="one_col")
            nc.vector.memset(eps_col, EPS)
            nc.vector.memset(nb_col, neg_bias)
            nc.vector.memset(one_col, 1.0)

            for kb in range(TBLK):
                for h in range(HPC):
                    nc.gpsimd.tensor_copy(vaug[kb][:, 65 * h + 64 : 65 * h + 65], one_col)

            # ---- phase 1+2: load xT/weights, projections with q/k rms-norm,
            #      and the v transposes into the vaug stationary tiles
            with tc.tile_pool(name="xw", bufs=1) as xw, \
                 tc.tile_pool(name="p2sb", bufs=4) as p2sb, \
                 tc.tile_pool(name="p2ps", bufs=5, space="PSUM") as p2ps, \
                 tc.tile_pool(name="p2pss", bufs=3, space="PSUM") as p2pss:
                w_sb = {}
                for nm in ("wv", "wq", "wk"):
                    w_sb[nm] = xw.tile([128, 2048], bf16, tag=f"{nm}_sb", name=f"{nm}_sb")
                xts = []
                for c in range(CB):
                    xt = xw.tile([128, T], bf16, tag=f"xt{c}", name=f"xt{c}")
                    xts.append(xt)

                # startup-critical order: wv, then x quarter 0, then wq/wk;
                # everything else trails behind the first matmuls.
                nc.sync.dma_start(w_sb["wv"], wv_d)
                for c in range(CB):
                    nc.sync.dma_start(
                        xts[c][:, 0:512], xT_d[128 * c : 128 * (c + 1), 0:512]
                    )
                nc.sync.dma_start(w_sb["wq"], wq_d)
                nc.sync.dma_start(sel2, sel2_d)
                nc.sync.dma_start(selT, selT_d)
                nc.sync.dma_start(ident, ident_d)
                nc.sync.dma_start(wq_col, wqc_d)
                nc.sync.dma_start(w_sb["wk"], wk_d)
                nc.sync.dma_start(wk_col, wkc_d)
                for nk in range(1, QT):
                    for c in range(CB):
                        nc.sync.dma_start(
                            xts[c][:, 512 * nk : 512 * (nk + 1)],
                            xT_d[128 * c : 128 * (c + 1), 512 * nk : 512 * (nk + 1)],
                        )
                    if nk == 1:
                        nc.sync.dma_start(ones_row, ones_row_d)
                        nc.sync.dma_start(pats, pats_d)
                    elif nk == 2:
                        nc.sync.dma_start(wo_sb, wo_d)

                for nk in range(QT):
                    worder = (
                        ("wv", False, vT, None),
                        ("wq", True, qT, wq_col),
                        ("wk", True, kT, wk_col),
                    )
                    if nk == QT - 1:
                        # put the short-epilogue v group last so the final
                        # psum users of this phase retire quickly and the
                        # attention pools can take over the banks sooner
                        worder = (worder[1], worder[2], worder[0])
                    for nm, isqk, outT, wcol in worder:
                        for g in range(NG):
                            cs = slice(512 * nk, 512 * (nk + 1))
                            pp = p2ps.tile([128, 512], f32, tag="proj", name="proj_ps")
                            for c in range(CB):
                                nc.tensor.matmul(
                                    pp,
                                    w_sb[nm][:, 256 * c + 128 * g : 256 * c + 128 * (g + 1)],
                                    xts[c][:, cs],
                                    start=(c == 0),
                                    stop=(c == CB - 1),
                                )
                            if not isqk:
                                # v: keep fp32->bf16 cast copy on Act (DVE is
                                # the busier engine in this phase)
                                nc.scalar.copy(vT[g][:, cs], pp)
                                for kb in range(4 * nk, 4 * nk + 4):
                                    pv = p2pss.tile([128, 128], bf16, tag="aux", name="vtr_ps")
                                    nc.tensor.transpose(
                                        pv,
                                        vT[g][:, 128 * kb : 128 * (kb + 1)],
                                        ident,
                                    )
                                    for hl in range(2):
                                        h = 2 * g + hl
                                        nc.scalar.copy(
                                            vaug[kb][:, 65 * h : 65 * h + 64],
                                            pv[:, 64 * hl : 64 * (hl + 1)],
                                        )
                                continue
                            sq = p2sb.tile([128, 512], bf16, tag="nrm", name="sq", bufs=10)
                            nc.scalar.activation(sq, pp, AF.Square)
                            st = p2pss.tile([2, 512], f32, tag="aux", name="st_ps")
                            nc.tensor.matmul(st, sel2, sq, start=True, stop=True)
                            # sqrt straight from PSUM (saves a copy), broadcast
                            # the rms row, then one reciprocal off PSUM
                            sts = p2sb.tile([2, 512], f32r, tag="nrm2", name="sts", bufs=6)
                            nc.scalar.activation(sts, st, AF.Sqrt, bias=eps_col[0:2], scale=1.0 / D)
                            mm = p2pss.tile([128, 512], f32, tag="aux", name="mm_ps")
                            nc.tensor.matmul(mm, selT, sts, start=True, stop=True)
                            mrb = p2sb.tile([128, 512], f32, tag="nrm", name="mrb", bufs=10)
                            nc.vector.reciprocal(mrb, mm)
                            nc.vector.scalar_tensor_tensor(
                                outT[g][:, cs], pp, wcol, mrb, OP.mult, OP.mult
                            )

            # ---- phase 4: attention (+ deferred per-q-tile output projection)
            with tc.tile_pool(name="atp", bufs=1) as atp, \
                 tc.tile_pool(name="p4sb", bufs=6) as p4sb, \
                 tc.tile_pool(name="p5sb", bufs=4) as p5sb, \
                 tc.tile_pool(name="psS", bufs=2, space="PSUM") as psS, \
                 tc.tile_pool(name="psO", bufs=4, space="PSUM") as psO:
                AT = [atp.tile([128, T], bf16, tag=f"AT{g}", name=f"AT{g}") for g in range(NG)]

                pending_oproj = []

                def emit_oproj():
                    while pending_oproj:
                        tb = pending_oproj.pop(0)
                        for nn in range(2):
                            po = psO.tile([128, 512], f32, tag="O", name="po_ps")
                            for cg in range(NG):
                                nc.tensor.matmul(
                                    po,
                                    AT[cg][:, 128 * tb : 128 * (tb + 1)],
                                    wo_sb[:, 1024 * cg + 512 * nn : 1024 * cg + 512 * (nn + 1)],
                                    start=(cg == 0),
                                    stop=(cg == NG - 1),
                                )
                            ob = p5sb.tile([128, 512], bf16, tag="ob", name="ob")
                            nc.vector.tensor_copy(ob, po)
                            nc.sync.dma_start(
                                out_d[128 * tb : 128 * (tb + 1), 512 * nn : 512 * (nn + 1)],
                                ob,
                            )

                for j in range(QT):
                    entries = plan[j]
                    if not entries:
                        continue
                    qs = slice(512 * j, 512 * (j + 1))
                    for g in range(NG):
                        o_ps = [
                            psO.tile([65, 512], f32, tag="O", name="o_ps")
                            for _ in range(2)
                        ]
                        nent = len(entries)
                        for ei, (kb, ql, qh, subs) in enumerate(entries):
                            s_ps = psS.tile([128, 1024], f32, tag="S", name="s_ps")
                            for hl in range(2):
                                rs = slice(64 * hl, 64 * (hl + 1))
                                nc.tensor.matmul(
                                    s_ps[:, 512 * hl + ql : 512 * hl + qh],
                                    kT[g][rs, 128 * kb : 128 * (kb + 1)],
                                    qT[g][rs, 512 * j + ql : 512 * j + qh],
                                    start=True,
                                    stop=True,
                                    tile_position=(64 * hl, 0),
                                )
                            pt = p4sb.tile([128, 1024], bf16, tag="PT", name="pt", bufs=8)
                            s3 = s_ps.rearrange("p (h w) -> p h w", h=2)
                            p3 = pt.rearrange("p (h w) -> p h w", h=2)
                            nc.scalar.activation(
                                p3[:, :, ql:qh],
                                s3[:, :, ql:qh],
                                AF.Exp,
                                bias=nb_col,
                                scale=SCALE,
                            )
                            for hl in range(2):
                                for qbl, pidx in subs:
                                    bs = slice(512 * hl + 128 * qbl, 512 * hl + 128 * (qbl + 1))
                                    nc.gpsimd.tensor_tensor(
                                        pt[:, bs],
                                        pt[:, bs],
                                        pats[:, 128 * pidx : 128 * (pidx + 1)],
                                        OP.mult,
                                    )
                            for hl in range(2):
                                h = 2 * g + hl
                                nc.tensor.matmul(
                                    o_ps[hl][:, ql:qh],
                                    vaug[kb][:, 65 * h : 65 * (h + 1)],
                                    pt[:, 512 * hl + ql : 512 * hl + qh],
                                    start=(ei == 0),
                                    stop=(ei == nent - 1),
                                )
                            if ei == 1:
                                # fill PE/psum slack mid-stream with the
                                # previous q-tile's output projection
                                emit_oproj()
                        for hl in range(2):
                            rs = slice(64 * hl, 64 * (hl + 1))
                            minv = p4sb.tile([1, 512], f32r, tag="ep", name="minv", bufs=6)
                            nc.vector.reciprocal(minv, o_ps[hl][64:65, :])
                            m2 = psO.tile([64, 512], f32, tag="O", name="m2_ps")
                            nc.tensor.matmul(
                                m2, ones_row[:, 0:64], minv, start=True, stop=True
                            )
                            m2c = p4sb.tile([64, 512], bf16, tag="ep2", name="m2c", bufs=4)
                            nc.vector.tensor_copy(m2c, m2)
                            if hl == 0:
                                nc.vector.tensor_tensor(
                                    AT[g][0:64, qs], o_ps[hl][0:64], m2c, OP.mult
                                )
                            else:
                                ab = p4sb.tile([64, 512], bf16, tag="ep2", name="ab", bufs=4)
                                nc.vector.tensor_tensor(ab, o_ps[hl][0:64], m2c, OP.mult)
                                nc.sync.dma_start(AT[g][64:128, qs], ab)
                            nc.vector.tensor_tensor(
                                AT[g][rs, qs], AT[g][rs, qs], vT[g][rs, qs], OP.add
                            )
                    pending_oproj.extend(range(4 * j, 4 * j + 4))
                emit_oproj()

    nc.compile()
    return nc


def kernel(**inputs):
    from concourse import bass_utils

    x = np.asarray(inputs["x"], np.float32)
    mask = np.asarray(inputs["attention_mask"])
    Wq = np.asarray(inputs["Wq"], np.float32)
    Wk = np.asarray(inputs["Wk"], np.float32)
    Wv = np.asarray(inputs["Wv"], np.float32)
    Wo = np.asarray(inputs["Wo"], np.float32)
    qw = np.asarray(inputs["q_norm_w"], np.float32)
    kw = np.asarray(inputs["k_norm_w"], np.float32)
    gate = np.asarray(inputs["gate"], np.float32).reshape(H)

    mask01 = mask.reshape(T, T) != 0
    plan, patterns = _analyze_mask(mask01)
    npat = patterns.shape[0]

    # fold the per-head gate into the value/output projections:
    # Wv_h /= gate_h and Wo_h *= gate_h leaves out = (attn*gate + vx) @ Wo
    # unchanged (the ones-column softmax denominators are gate-free).
    Wv = Wv / np.repeat(gate, D)[None, :]
    Wo = Wo * np.repeat(gate, D)[:, None]

    # if |score| can get near exp overflow, shift by a constant (cancels in
    # the softmax normalization)
    bound = 8.0 * np.max(np.abs(qw)) * np.max(np.abs(kw))
    neg_bias = -max(0.0, float(bound) - 60.0)

    key = (hash(mask01.tobytes()), npat, neg_bias)
    if key not in _CACHE:
        _CACHE[key] = _build_program(plan, npat, neg_bias)
    nc = _CACHE[key]

    pats_r = np.ascontiguousarray(
        patterns.transpose(1, 0, 2).reshape(128, 128 * npat)
    ).astype(BF16)
    ones_row = np.ones((1, 128), np.float32)
    sel2 = np.zeros((128, 2), np.float32)
    sel2[0:64, 0] = 1.0
    sel2[64:128, 1] = 1.0
    selT = np.ascontiguousarray(sel2.T)
    ident128 = np.eye(128, dtype=np.float32).astype(BF16)
    wq_col = np.tile(qw, 2)[:, None].astype(np.float32)
    wk_col = np.tile(kw, 2)[:, None].astype(np.float32)

    def chunk_major(W):
        # [C_in, N] -> [128, (C_in/128)*N] with 128-row chunks side by side
        ci, n = W.shape
        return np.ascontiguousarray(
            W.reshape(ci // 128, 128, n).transpose(1, 0, 2).reshape(128, -1)
        ).astype(BF16)

    in_maps = []
    for core in range(NCORES):
        b, grp = core // 4, core % 4
        hs = slice(256 * grp, 256 * (grp + 1))
        in_maps.append(
            {
                "xT": np.ascontiguousarray(x[b].T).astype(BF16),
                "wq": chunk_major(Wq[:, hs]),
                "wk": chunk_major(Wk[:, hs]),
                "wv": chunk_major(Wv[:, hs]),
                "wo": chunk_major(Wo[hs, :]),
                "wq_col": wq_col,
                "wk_col": wk_col,
                "ones_row": ones_row,
                "sel2": sel2.astype(BF16),
                "selT": selT,
                "ident128": ident128,
                "pats": pats_r,
            }
        )

    global _LAST_IN_MAPS
    _LAST_IN_MAPS = in_maps
    res = bass_utils.run_bass_kernel_spmd(nc, in_maps, core_ids=list(range(NCORES)))
    parts = [res.results[i]["out"].astype(np.float32) for i in range(NCORES)]
    out = np.stack(
        [
            parts[0] + parts[1] + parts[2] + parts[3],
            parts[4] + parts[5] + parts[6] + parts[7],
        ]
    )
    return out.astype(np.float32)
